# revision 1
# baseline (speedup 1.0000x reference)
"""Trainium2 Bass kernel for nn_DecoderRNN: 64-step 2-layer tanh RNN + per-step FC.

Sharding (8 cores, no collectives):
  - 2-way data parallel over batch (cores 0-3: rows 0:128, cores 4-7: rows 128:256).
    Each group of 4 cores redundantly computes its batch-half's RNN (the RNN is
    1/3 of total FLOPs; replication buys full M=128 PE utilization).
  - 4-way tensor parallel over the fc output dim (O=8192 -> 2048 per core).

Numerics: all matmuls in bf16 with fp32 PSUM accumulation (bf16 x bf16
products are exact in fp32). RNN weights and the recurrent state are split
hi/lo (x = hi + lo, both bf16), and each GEMM runs three passes
(hi@Whi + hi@Wlo + lo@Whi), giving ~16-bit effective mantissa through the
64-step recurrence. The FC is single-pass bf16 (logits are not recurrent).
Measured end-to-end rel err ~2.3e-3 vs the fp32 reference.

Per-core compute, per step t (B=128, H=1024, O_slice=2048):
  - State kept transposed ("g" form, [H, B]): g tiles are the matmul stationary
    operand (lhsT), weights stream as the moving operand at N=512.
  - Bias enters the PSUM accumulation via K=1 matmuls against a ones row
    (hi+lo split as well). tanh on the scalar engine in fp32.
  - Layer outputs ([B, H] in PSUM) are transposed back to g form on the tensor
    engine (4 transposed tiles per PSUM bank, drained by wide DVE copies that
    also perform the hi/lo split).
  - logits copied PSUM->SBUF->HBM per step.
"""
import sys

sys.path.insert(0, "/opt/trn_rl_repo")

from contextlib import ExitStack

import numpy as np
import ml_dtypes

import concourse.bass as bass
import concourse.tile as tile
from concourse import bacc, mybir
from concourse.bass_utils import run_bass_kernel_spmd

H = 1024
O = 8192
L = 2
T = 64
B = 256
N_CORES = 8
BG = B // 2          # batch rows per core (2-way DP)
OS = O // 4          # fc output slice per core (4-way TP)
KT = H // 128        # 8 k-tiles per 1024 contraction
F32 = mybir.dt.float32
BF16 = mybir.dt.bfloat16

_cached = {}

RNN_W_NAMES = ["ih0", "hh0", "ih1", "hh1"]


def _build_program(n_steps: int, n_reps: int = 1, skip_fc=False, skip_tr=False,
                   skip_rnn=False):
    nc = bacc.Bacc("TRN2", target_bir_lowering=False, debug=False, num_devices=N_CORES)

    # --- DRAM parameters (per-core shards, host-prepared layouts) ---
    # RNN weights, transposed+tiled+hi/lo-split on host: [p][k][n] of W.T
    wd = {}
    for nm in RNN_W_NAMES:
        for part in ("h", "l"):
            wd[nm + part] = nc.declare_dram_parameter(
                f"w_{nm}{part}", [128, KT, H], BF16, isOutput=False)
    w_fc = nc.declare_dram_parameter("w_fc", [128, KT, OS], BF16, isOutput=False)
    # initial state, g form hi/lo: [p][k][b] = state[b, k*128+p]
    gd = {}
    for nm in ("x", "h0", "h1"):
        for part in ("h", "l"):
            gd[nm + part] = nc.declare_dram_parameter(
                f"g_{nm}{part}", [128, KT, BG], BF16, isOutput=False)
    # bias rows (b_ih + b_hh per layer, hi/lo), fc bias slice, ones row, identity
    bd = {}
    for l in range(L):
        for part in ("h", "l"):
            bd[f"b{l}{part}"] = nc.declare_dram_parameter(
                f"b{l}{part}", [1, H], BF16, isOutput=False)
    fcbd = nc.declare_dram_parameter("fcb", [1, OS], BF16, isOutput=False)
    onesbd = nc.declare_dram_parameter("onesb", [1, 128], BF16, isOutput=False)
    identd = nc.declare_dram_parameter("ident", [128, 128], F32, isOutput=False)

    out_d = nc.declare_dram_parameter("out", [n_steps, 128, OS], F32, isOutput=True)

    with tile.TileContext(nc) as tc, ExitStack() as ctx:
        wpool = ctx.enter_context(tc.tile_pool(name="w", bufs=1))
        cpool = ctx.enter_context(tc.tile_pool(name="c", bufs=1))
        ghp = ctx.enter_context(tc.tile_pool(name="ghp", bufs=3))
        glp = ctx.enter_context(tc.tile_pool(name="glp", bufs=3))
        hp = ctx.enter_context(tc.tile_pool(name="h", bufs=2))
        logp = ctx.enter_context(tc.tile_pool(name="log", bufs=2))
        rnn_ps = ctx.enter_context(tc.tile_pool(name="rnnps", bufs=1, space="PSUM"))
        tr_ps = ctx.enter_context(tc.tile_pool(name="trps", bufs=2, space="PSUM"))
        fc_ps = ctx.enter_context(tc.tile_pool(name="fcps", bufs=1, space="PSUM"))

        # --- preamble: load weights/constants ---
        w = {}
        for nm, dram in wd.items():
            t_ = wpool.tile([128, KT, H], BF16, tag=f"w{nm}", name=f"w{nm}")
            nc.sync.dma_start(t_[:], dram[:])
            w[nm] = t_
        wfc = wpool.tile([128, KT, OS], BF16, tag="wfc")
        nc.sync.dma_start(wfc[:], w_fc[:])

        bb = {}
        for nm, dram in bd.items():
            t_ = cpool.tile([1, H], BF16, tag=nm, name=nm)
            nc.sync.dma_start(t_[:], dram[:])
            bb[nm] = t_
        fcb = cpool.tile([1, OS], BF16, tag="fcb")
        onesb = cpool.tile([1, 128], BF16, tag="onesb")
        ident = cpool.tile([128, 128], F32, tag="ident")
        for t_, d_ in [(fcb, fcbd), (onesb, onesbd), (ident, identd)]:
            nc.sync.dma_start(t_[:], d_[:])

        # --- initial state ---
        def g_init(nm):
            gh = ghp.tile([128, KT, BG], BF16, tag="gh", name="gh")
            gl = glp.tile([128, KT, BG], BF16, tag="gl", name="gl")
            nc.sync.dma_start(gh[:], gd[nm + "h"][:])
            nc.sync.dma_start(gl[:], gd[nm + "l"][:])
            return gh, gl

        g_x = g_init("x")
        g_h0 = g_init("h0")
        g_h1 = g_init("h1")

        def rnn_layer(g_in, g_h, w_inh, w_inl, w_hh_, w_hl_, bh, bl):
            """tanh(in @ W_ihT + h @ W_hhT + b) -> h_sbuf [128(B), H] fp32.

            3-pass hi/lo bf16 per operand: hi@Whi + hi@Wlo + lo@Whi."""
            inh, inl = g_in
            hh, hl = g_h
            ps = rnn_ps.tile([128, H], F32, tag="rnnps")
            for nck in range(2):
                nsl = bass.ts(nck, 512)
                nc.tensor.matmul(ps[:, nsl], onesb[:, :], bh[:, nsl],
                                 start=True, stop=False)
                nc.tensor.matmul(ps[:, nsl], onesb[:, :], bl[:, nsl],
                                 start=False, stop=False)
                passes = ((inh, w_inh), (inh, w_inl), (inl, w_inh),
                          (hh, w_hh_), (hh, w_hl_), (hl, w_hh_))
                for pi, (lhs, rhs) in enumerate(passes):
                    for k in range(KT):
                        last = pi == len(passes) - 1 and k == KT - 1
                        nc.tensor.matmul(ps[:, nsl], lhs[:, k, :], rhs[:, k, nsl],
                                         start=False, stop=last)
            h_sb = hp.tile([128, H], F32, tag="h")
            nc.scalar.activation(h_sb[:], ps[:], mybir.ActivationFunctionType.Tanh)
            return h_sb

        def to_g(h_sb):
            """PE-transpose [B, H] -> g form [H(p), B], split hi/lo bf16.

            4 transposed 128x128 tiles per PSUM bank; each bank drained by a
            wide DVE copy (bf16 rounding -> hi) plus a subtract (-> lo)."""
            gh = ghp.tile([128, KT, BG], BF16, tag="gh", name="gh")
            gl = glp.tile([128, KT, BG], BF16, tag="gl", name="gl")
            for grp in range(2):
                pt = tr_ps.tile([128, 512], F32, tag="trps", name="pt")
                for j in range(4):
                    k = grp * 4 + j
                    nc.tensor.transpose(pt[:, bass.ts(j, 128)],
                                        h_sb[:, bass.ts(k, 128)], ident[:])
                ghs = gh[:, grp * 4:(grp + 1) * 4, :]
                gls = gl[:, grp * 4:(grp + 1) * 4, :]
                nc.vector.tensor_copy(ghs, pt[:])
                nc.vector.tensor_sub(gls, pt[:], ghs)
            return gh, gl

        for t in range(n_steps * n_reps):
            t = t % n_steps
            if not skip_rnn:
                h0_sb = rnn_layer(g_x, g_h0, w["ih0h"], w["ih0l"], w["hh0h"],
                                  w["hh0l"], bb["b0h"], bb["b0l"])
                if not skip_tr:
                    g_h0 = to_g(h0_sb)
                h1_sb = rnn_layer(g_h0, g_h1, w["ih1h"], w["ih1l"], w["hh1h"],
                                  w["hh1l"], bb["b1h"], bb["b1l"])
                if not skip_tr:
                    g_h1 = to_g(h1_sb)
                g_x = g_h1
            g_fc = g_h1[0]  # hi part, bf16 — FC lhsT

            if skip_fc:
                continue
            # FC: logits[B, OS] = h1 @ fc_W_slice.T + fc_b_slice   (bf16)
            # two [128, 1024] halves to keep PSUM at 2 banks
            for half in range(2):
                ps = fc_ps.tile([128, OS // 2], F32, tag="fcps", name="fps")
                for nck in range(2):
                    fsl = bass.ts(half * 2 + nck, 512)   # slice into wfc/fcb
                    nsl = bass.ts(nck, 512)              # slice into ps
                    nc.tensor.matmul(ps[:, nsl], onesb[:, :], fcb[:, fsl],
                                     start=True, stop=False)
                    for k in range(KT):
                        nc.tensor.matmul(ps[:, nsl], g_fc[:, k, :], wfc[:, k, fsl],
                                         start=False, stop=(k == KT - 1))
                lsb = logp.tile([128, OS // 2], F32, tag="log", name="lsb")
                nc.vector.tensor_copy(lsb[:], ps[:])
                nc.sync.dma_start(out_d[t][:, bass.ts(half, OS // 2)], lsb[:])

    nc.finalize()
    return nc


def _hi(a):
    return a.astype(ml_dtypes.bfloat16)


def _lo(a):
    return (a.astype(np.float32)
            - a.astype(ml_dtypes.bfloat16).astype(np.float32)).astype(
        ml_dtypes.bfloat16)


def _prep_inputs(x, hidden, W_ih, W_hh, b_ih, b_hh, fc_W, fc_b, n_steps):
    """Build the 8 per-core input maps (host-side transposes / hi-lo split)."""
    def gform(a):  # [BG, H] f32 -> [128, KT, BG]: out[p, k, b] = a[b, k*128+p]
        return np.ascontiguousarray(
            a.T.reshape(KT, 128, BG).transpose(1, 0, 2)).astype(np.float32)

    def wform(Wmat):  # [H_out, H_in] -> [128, KT, H_out] of W.T (f32)
        return np.ascontiguousarray(
            Wmat.T.reshape(KT, 128, Wmat.shape[0]).transpose(1, 0, 2)).astype(
                np.float32)

    ident = np.eye(128, dtype=np.float32)
    onesb = np.ones((1, 128), ml_dtypes.bfloat16)

    common = {"onesb": onesb, "ident": ident}
    for l, nm_pair in enumerate([("ih0", "hh0"), ("ih1", "hh1")]):
        for nm, Wmat in zip(nm_pair, (W_ih[l], W_hh[l])):
            wt = wform(Wmat)
            common[f"w_{nm}h"] = _hi(wt)
            common[f"w_{nm}l"] = _lo(wt)
        brow = (b_ih[l] + b_hh[l]).astype(np.float32).reshape(1, H)
        common[f"b{l}h"] = _hi(brow)
        common[f"b{l}l"] = _lo(brow)

    in_maps = []
    for c in range(N_CORES):
        bg, j = c // 4, c % 4
        bsl = slice(bg * BG, (bg + 1) * BG)
        osl = slice(j * OS, (j + 1) * OS)
        wfc = _hi(np.ascontiguousarray(
            fc_W[osl].T.reshape(KT, 128, OS).transpose(1, 0, 2)).astype(np.float32))
        m = dict(common)
        m["w_fc"] = wfc
        m["fcb"] = fc_b[osl].astype(ml_dtypes.bfloat16).reshape(1, OS)
        for nm, src in (("x", x[0, bsl]), ("h0", hidden[0, bsl]),
                        ("h1", hidden[1, bsl])):
            g = gform(src)
            m[f"g_{nm}h"] = _hi(g)
            m[f"g_{nm}l"] = _lo(g)
        in_maps.append(m)
    return in_maps


def kernel(x, hidden, embedded, W_ih, W_hh, b_ih, b_hh, fc_W, fc_b,
           _trace=False, _trace_kwargs=None):
    n_steps = embedded.shape[0]
    key = n_steps
    if key not in _cached:
        _cached[key] = _build_program(n_steps)
    nc = _cached[key]

    in_maps = _prep_inputs(np.asarray(x), np.asarray(hidden), np.asarray(W_ih),
                           np.asarray(W_hh), np.asarray(b_ih), np.asarray(b_hh),
                           np.asarray(fc_W), np.asarray(fc_b), n_steps)
    core_ids = list(range(N_CORES))
    res = run_bass_kernel_spmd(nc, in_maps, core_ids, trace=_trace,
                               **(_trace_kwargs or {}))

    out = np.empty((n_steps, 1, B, O), np.float32)
    for c in range(N_CORES):
        bg, j = c // 4, c % 4
        out[:, 0, bg * BG:(bg + 1) * BG, j * OS:(j + 1) * OS] = res.results[c]["out"]
    if _trace:
        kernel.last_results = res
    return out



# revision 2
# speedup vs baseline: 1.1461x; 1.1461x over previous
"""Trainium2 Bass kernel for nn_DecoderRNN: 64-step 2-layer tanh RNN + per-step FC.

Sharding (8 cores, no collectives):
  - 2-way data parallel over batch (cores 0-3: rows 0:128, cores 4-7: rows 128:256).
  - 4-way tensor parallel over the fc output dim (O=8192 -> 2048 per core).

Numerics: RNN GEMMs run in float32r (fp32 operands truncated to FP22 by the PE,
13-bit mantissa) — a single pass at bf16 speed for moving dim >= 256. Simulated
end-to-end rel err ~3e-3 vs the fp32 reference (the bf16 FC contributes ~2.3e-3;
fp22 RNN truncation ~1e-3 after 64-step error growth).

Per-core compute, per step t (B=128, H=1024, O_slice=2048):
  - State kept transposed ("g" form, [H, B] f32): g tiles are the matmul
    stationary operand (lhsT), weights stream as the moving operand at N=512.
  - Bias enters the PSUM accumulation via K=1 matmuls against a ones row.
  - tanh on the scalar engine; layer outputs transposed back to g form on the
    tensor engine (f32r transposes), drained by DVE copies.
  - FC in bf16 (lhsT = bf16 copy of g_h1) interleaved into the NEXT step's
    dependency gaps: issue order per step is
      L0(bias+hh), L0(in), L1(bias+hh), tr(h0), FC(t-1) half0, L1(in),
      FC(t-1) half1, tr(h1)
    so the PE never waits on tanh/DVE drains.
"""
import sys

sys.path.insert(0, "/opt/trn_rl_repo")

from contextlib import ExitStack

import numpy as np
import ml_dtypes

import concourse.bass as bass
import concourse.tile as tile
from concourse import bacc, mybir
from concourse.bass_utils import run_bass_kernel_spmd

H = 1024
O = 8192
L = 2
T = 64
B = 256
N_CORES = 8
BG = B // 2          # batch rows per core (2-way DP)
OS = O // 4          # fc output slice per core (4-way TP)
KT = H // 128        # 8 k-tiles per 1024 contraction
F32 = mybir.dt.float32
F32R = mybir.dt.float32r
BF16 = mybir.dt.bfloat16

_cached = {}


def _build_program(n_steps: int):
    nc = bacc.Bacc("TRN2", target_bir_lowering=False, debug=False, num_devices=N_CORES)

    # --- DRAM parameters (per-core shards, host-prepared layouts) ---
    # RNN weights, transposed+tiled on host: [p][k][n] of W.T, f32
    wd = {}
    for nm in ("ih0", "hh0", "ih1", "hh1"):
        wd[nm] = nc.declare_dram_parameter(f"w_{nm}", [128, KT, H], F32R,
                                           isOutput=False)
    w_fc = nc.declare_dram_parameter("w_fc", [128, KT, OS], BF16, isOutput=False)
    # initial state, g form: [p][k][b] = state[b, k*128+p], f32
    gd = {}
    for nm in ("x", "h0", "h1"):
        gd[nm] = nc.declare_dram_parameter(f"g_{nm}", [128, KT, BG], F32R,
                                           isOutput=False)
    bd = {}
    for l in range(L):
        bd[l] = nc.declare_dram_parameter(f"b{l}", [1, H], F32R, isOutput=False)
    fcbd = nc.declare_dram_parameter("fcb", [1, OS], BF16, isOutput=False)
    onesfd = nc.declare_dram_parameter("onesf", [1, 128], F32R, isOutput=False)
    onesbd = nc.declare_dram_parameter("onesb", [1, 128], BF16, isOutput=False)
    identd = nc.declare_dram_parameter("ident", [128, 128], F32R, isOutput=False)

    out_d = nc.declare_dram_parameter("out", [n_steps, 128, OS], F32, isOutput=True)

    with tile.TileContext(nc) as tc, ExitStack() as ctx:
        wpool = ctx.enter_context(tc.tile_pool(name="w", bufs=1))
        cpool = ctx.enter_context(tc.tile_pool(name="c", bufs=1))
        gp = ctx.enter_context(tc.tile_pool(name="gp", bufs=3))
        gfcp = ctx.enter_context(tc.tile_pool(name="gfc", bufs=2))
        hp = ctx.enter_context(tc.tile_pool(name="h", bufs=2))
        logp = ctx.enter_context(tc.tile_pool(name="log", bufs=2))
        rnn_ps = ctx.enter_context(tc.tile_pool(name="rnnps", bufs=2, space="PSUM"))
        tr_ps = ctx.enter_context(tc.tile_pool(name="trps", bufs=2, space="PSUM"))
        fc_ps = ctx.enter_context(tc.tile_pool(name="fcps", bufs=1, space="PSUM"))

        # --- preamble: load weights/constants ---
        w = {}
        for nm, dram in wd.items():
            t_ = wpool.tile([128, KT, H], F32R, tag=f"w{nm}", name=f"w{nm}")
            nc.sync.dma_start(t_[:], dram[:])
            w[nm] = t_
        wfc = wpool.tile([128, KT, OS], BF16, tag="wfc")
        nc.sync.dma_start(wfc[:], w_fc[:])

        bb = {}
        for l, dram in bd.items():
            t_ = cpool.tile([1, H], F32R, tag=f"b{l}", name=f"b{l}")
            nc.sync.dma_start(t_[:], dram[:])
            bb[l] = t_
        fcb = cpool.tile([1, OS], BF16, tag="fcb")
        onesf = cpool.tile([1, 128], F32R, tag="onesf")
        onesb = cpool.tile([1, 128], BF16, tag="onesb")
        ident = cpool.tile([128, 128], F32R, tag="ident")
        for t_, d_ in [(fcb, fcbd), (onesf, onesfd), (onesb, onesbd),
                       (ident, identd)]:
            nc.sync.dma_start(t_[:], d_[:])

        # --- initial state ---
        def g_init(nm):
            g = gp.tile([128, KT, BG], F32R, tag="g", name=f"g{nm}")
            nc.sync.dma_start(g[:], gd[nm][:])
            return g

        g_x = g_init("x")
        g_h0 = g_init("h0")
        g_h1 = g_init("h1")

        def gemm_open(ps, g_h, w_hh_, brow):
            """bias + hidden-state half of the layer GEMM (no deps on this
            step's earlier output; fills PE while tanh/drains run)."""
            for nck in range(2):
                nsl = bass.ts(nck, 512)
                nc.tensor.matmul(ps[:, nsl], onesf[:, :], brow[:, nsl],
                                 start=True, stop=False)
                for k in range(KT):
                    nc.tensor.matmul(ps[:, nsl], g_h[:, k, :], w_hh_[:, k, nsl],
                                     start=False, stop=False)

        def gemm_close(ps, g_in, w_in):
            """input half; closes both accumulation groups."""
            for nck in range(2):
                nsl = bass.ts(nck, 512)
                for k in range(KT):
                    nc.tensor.matmul(ps[:, nsl], g_in[:, k, :], w_in[:, k, nsl],
                                     start=False, stop=(k == KT - 1))

        def tanh_h(ps):
            h_sb = hp.tile([128, H], F32R, tag="h")
            nc.scalar.activation(h_sb[:], ps[:], mybir.ActivationFunctionType.Tanh)
            return h_sb

        def to_g(h_sb):
            """PE-transpose [B, H] -> g form [H(p), B] f32r."""
            g = gp.tile([128, KT, BG], F32R, tag="g", name="g")
            for grp in range(2):
                pt = tr_ps.tile([128, 512], F32R, tag="trps", name="pt")
                for j in range(4):
                    k = grp * 4 + j
                    nc.tensor.transpose(pt[:, bass.ts(j, 128)],
                                        h_sb[:, bass.ts(k, 128)], ident[:])
                nc.vector.tensor_copy(g[:, grp * 4:(grp + 1) * 4, :], pt[:])
            return g

        def fc_half(t, g_fc, half):
            """logits[:, half] = h1 @ fc_W_slice.T + fc_b_slice   (bf16)"""
            ps = fc_ps.tile([128, OS // 2], F32, tag="fcps", name="fps")
            for nck in range(2):
                fsl = bass.ts(half * 2 + nck, 512)   # slice into wfc/fcb
                nsl = bass.ts(nck, 512)              # slice into ps
                nc.tensor.matmul(ps[:, nsl], onesb[:, :], fcb[:, fsl],
                                 start=True, stop=False)
                for k in range(KT):
                    nc.tensor.matmul(ps[:, nsl], g_fc[:, k, :], wfc[:, k, fsl],
                                     start=False, stop=(k == KT - 1))
            lsb = logp.tile([128, OS // 2], F32, tag="log", name="lsb")
            nc.vector.tensor_copy(lsb[:], ps[:])
            nc.sync.dma_start(out_d[t][:, bass.ts(half, OS // 2)], lsb[:])

        g_fc_prev = None
        for t in range(n_steps):
            ps0 = rnn_ps.tile([128, H], F32, tag="rnnps", name="ps0")
            gemm_open(ps0, g_h0, w["hh0"], bb[0])
            gemm_close(ps0, g_x, w["ih0"])
            h0_sb = tanh_h(ps0)

            ps1 = rnn_ps.tile([128, H], F32, tag="rnnps", name="ps1")
            gemm_open(ps1, g_h1, w["hh1"], bb[1])   # PE: fills tanh(h0) wait
            g_h0 = to_g(h0_sb)
            if t > 0:
                fc_half(t - 1, g_fc_prev, 0)        # PE: fills g_h0 drain wait
            gemm_close(ps1, g_h0, w["ih1"])
            h1_sb = tanh_h(ps1)
            if t > 0:
                fc_half(t - 1, g_fc_prev, 1)        # PE: fills tanh(h1) wait
            g_h1 = to_g(h1_sb)
            g_x = g_h1
            g_fc = gfcp.tile([128, KT, BG], BF16, tag="gfc", name="gfc")
            nc.vector.tensor_copy(g_fc[:], g_h1[:])
            g_fc_prev = g_fc

        fc_half(n_steps - 1, g_fc_prev, 0)
        fc_half(n_steps - 1, g_fc_prev, 1)

    nc.finalize()
    return nc


def _prep_inputs(x, hidden, W_ih, W_hh, b_ih, b_hh, fc_W, fc_b, n_steps):
    """Build the 8 per-core input maps (host-side transposes)."""
    def gform(a):  # [BG, H] f32 -> [128, KT, BG]: out[p, k, b] = a[b, k*128+p]
        return np.ascontiguousarray(
            a.T.reshape(KT, 128, BG).transpose(1, 0, 2)).astype(np.float32)

    def wform(Wmat):  # [H_out, H_in] -> [128, KT, H_out] of W.T (f32)
        return np.ascontiguousarray(
            Wmat.T.reshape(KT, 128, Wmat.shape[0]).transpose(1, 0, 2)).astype(
                np.float32)

    common = {
        "onesf": np.ones((1, 128), np.float32),
        "onesb": np.ones((1, 128), ml_dtypes.bfloat16),
        "ident": np.eye(128, dtype=np.float32),
    }
    for l, nm_pair in enumerate([("ih0", "hh0"), ("ih1", "hh1")]):
        for nm, Wmat in zip(nm_pair, (W_ih[l], W_hh[l])):
            common[f"w_{nm}"] = wform(Wmat)
        common[f"b{l}"] = (b_ih[l] + b_hh[l]).astype(np.float32).reshape(1, H)

    in_maps = []
    for c in range(N_CORES):
        bg, j = c // 4, c % 4
        bsl = slice(bg * BG, (bg + 1) * BG)
        osl = slice(j * OS, (j + 1) * OS)
        wfc = np.ascontiguousarray(
            fc_W[osl].T.reshape(KT, 128, OS).transpose(1, 0, 2)).astype(
                ml_dtypes.bfloat16)
        m = dict(common)
        m["w_fc"] = wfc
        m["fcb"] = fc_b[osl].astype(ml_dtypes.bfloat16).reshape(1, OS)
        for nm, src in (("x", x[0, bsl]), ("h0", hidden[0, bsl]),
                        ("h1", hidden[1, bsl])):
            m[f"g_{nm}"] = gform(src)
        in_maps.append(m)
    return in_maps


def kernel(x, hidden, embedded, W_ih, W_hh, b_ih, b_hh, fc_W, fc_b,
           _trace=False, _trace_kwargs=None):
    n_steps = embedded.shape[0]
    key = n_steps
    if key not in _cached:
        _cached[key] = _build_program(n_steps)
    nc = _cached[key]

    in_maps = _prep_inputs(np.asarray(x), np.asarray(hidden), np.asarray(W_ih),
                           np.asarray(W_hh), np.asarray(b_ih), np.asarray(b_hh),
                           np.asarray(fc_W), np.asarray(fc_b), n_steps)
    core_ids = list(range(N_CORES))
    res = run_bass_kernel_spmd(nc, in_maps, core_ids, trace=_trace,
                               **(_trace_kwargs or {}))

    out = np.empty((n_steps, 1, B, O), np.float32)
    for c in range(N_CORES):
        bg, j = c // 4, c % 4
        out[:, 0, bg * BG:(bg + 1) * BG, j * OS:(j + 1) * OS] = res.results[c]["out"]
    if _trace:
        kernel.last_results = res
    return out


# revision 3
# speedup vs baseline: 1.1624x; 1.0142x over previous
"""Trainium2 Bass kernel for nn_DecoderRNN: 64-step 2-layer tanh RNN + per-step FC.

Sharding (8 cores, no collectives):
  - 2-way data parallel over batch (cores 0-3: rows 0:128, cores 4-7: rows 128:256).
  - 4-way tensor parallel over the fc output dim (O=8192 -> 2048 per core).

Numerics: RNN GEMMs in float32r (PE truncates operands to FP22) — single pass at
full PE speed for moving dim >= 512. Measured rel err ~5e-3 vs fp32 reference.

v3 structure (vs v2): zero bias matmuls on the PE.
  - RNN bias+tanh fused into the scalar engine: the layer GEMM accumulates
    in @ W_ih.T + h @ W_hh.T in PSUM [B, H]; DVE copies the raw preact to SBUF;
    PE transposes it to [H(p), B]; scalar then applies tanh(x + b) per 128-row
    k-tile, where the bias is a per-partition scalar AP — writing the g-form
    state directly.
  - FC bias folded into the DVE PSUM drain (tensor_add against a
    host-pre-broadcast [128, OS] bias tile).
Per-step PE work: 64 RNN matmuls (f32r N=512), 32 FC matmuls (bf16 N=512),
16 transposes. FC(t-1) is interleaved into step t's dependency gaps.
"""
import sys

sys.path.insert(0, "/opt/trn_rl_repo")

from contextlib import ExitStack

import numpy as np
import ml_dtypes

import concourse.bass as bass
import concourse.tile as tile
from concourse import bacc, mybir
from concourse.bass_utils import run_bass_kernel_spmd

H = 1024
O = 8192
L = 2
T = 64
B = 256
N_CORES = 8
BG = B // 2          # batch rows per core (2-way DP)
OS = O // 4          # fc output slice per core (4-way TP)
KT = H // 128        # 8 k-tiles per 1024 contraction
F32 = mybir.dt.float32
F32R = mybir.dt.float32r
BF16 = mybir.dt.bfloat16

_cached = {}


def _build_program(n_steps: int):
    nc = bacc.Bacc("TRN2", target_bir_lowering=False, debug=False, num_devices=N_CORES)

    # --- DRAM parameters (per-core shards, host-prepared layouts) ---
    wd = {}
    for nm in ("ih0", "hh0", "ih1", "hh1"):
        wd[nm] = nc.declare_dram_parameter(f"w_{nm}", [128, KT, H], F32R,
                                           isOutput=False)
    w_fc = nc.declare_dram_parameter("w_fc", [128, KT, OS], BF16, isOutput=False)
    gd = {}
    for nm in ("x", "h0", "h1"):
        gd[nm] = nc.declare_dram_parameter(f"g_{nm}", [128, KT, BG], F32R,
                                           isOutput=False)
    # biases in g-layout: bg[p, l, k] = (b_ih + b_hh)[l][k*128 + p]
    bgd = nc.declare_dram_parameter("bg", [128, L, KT], F32, isOutput=False)
    # fc bias pre-broadcast across partitions
    fcbd = nc.declare_dram_parameter("fcbb", [128, OS], BF16, isOutput=False)
    identd = nc.declare_dram_parameter("ident", [128, 128], F32R, isOutput=False)

    out_d = nc.declare_dram_parameter("out", [n_steps, 128, OS], F32, isOutput=True)

    with tile.TileContext(nc) as tc, ExitStack() as ctx:
        wpool = ctx.enter_context(tc.tile_pool(name="w", bufs=1))
        cpool = ctx.enter_context(tc.tile_pool(name="c", bufs=1))
        gp = ctx.enter_context(tc.tile_pool(name="gp", bufs=3))
        gfcp = ctx.enter_context(tc.tile_pool(name="gfc", bufs=2))
        hp = ctx.enter_context(tc.tile_pool(name="h", bufs=2))
        logp = ctx.enter_context(tc.tile_pool(name="log", bufs=2))
        rnn_ps = ctx.enter_context(tc.tile_pool(name="rnnps", bufs=2, space="PSUM"))
        tr_ps = ctx.enter_context(tc.tile_pool(name="trps", bufs=2, space="PSUM"))
        fc_ps = ctx.enter_context(tc.tile_pool(name="fcps", bufs=1, space="PSUM"))

        # --- preamble: load weights/constants ---
        w = {}
        for nm, dram in wd.items():
            t_ = wpool.tile([128, KT, H], F32R, tag=f"w{nm}", name=f"w{nm}")
            nc.sync.dma_start(t_[:], dram[:])
            w[nm] = t_
        wfc = wpool.tile([128, KT, OS], BF16, tag="wfc")
        nc.sync.dma_start(wfc[:], w_fc[:])

        bg = cpool.tile([128, L, KT], F32, tag="bg")
        fcbb = cpool.tile([128, OS], BF16, tag="fcbb")
        ident = cpool.tile([128, 128], F32R, tag="ident")
        for t_, d_ in [(bg, bgd), (fcbb, fcbd), (ident, identd)]:
            nc.sync.dma_start(t_[:], d_[:])

        # --- initial state ---
        def g_init(nm):
            g = gp.tile([128, KT, BG], F32R, tag="g", name=f"g{nm}")
            nc.sync.dma_start(g[:], gd[nm][:])
            return g

        g_x = g_init("x")
        g_h0 = g_init("h0")
        g_h1 = g_init("h1")

        def gemm_open(ps, g_h, w_hh_):
            """hidden-state half of the layer GEMM (no deps on this step's
            earlier output; fills PE while tanh/drains run)."""
            for nck in range(2):
                nsl = bass.ts(nck, 512)
                for k in range(KT):
                    nc.tensor.matmul(ps[:, nsl], g_h[:, k, :], w_hh_[:, k, nsl],
                                     start=(k == 0), stop=False)

        def gemm_close(ps, g_in, w_in):
            """input half; closes both accumulation groups."""
            for nck in range(2):
                nsl = bass.ts(nck, 512)
                for k in range(KT):
                    nc.tensor.matmul(ps[:, nsl], g_in[:, k, :], w_in[:, k, nsl],
                                     start=False, stop=(k == KT - 1))

        def drain_layer(ps, l):
            """preact PSUM [B, H] -> g form [H(p), B] with tanh(x+b) on scalar."""
            h_pre = hp.tile([128, H], F32R, tag="h")
            nc.vector.tensor_copy(h_pre[:], ps[:])
            g = gp.tile([128, KT, BG], F32R, tag="g", name="g")
            for grp in range(2):
                pt = tr_ps.tile([128, 512], F32R, tag="trps", name="pt")
                for j in range(4):
                    k = grp * 4 + j
                    nc.tensor.transpose(pt[:, bass.ts(j, 128)],
                                        h_pre[:, bass.ts(k, 128)], ident[:])
                for j in range(4):
                    k = grp * 4 + j
                    nc.scalar.activation(g[:, k, :], pt[:, bass.ts(j, 128)],
                                         mybir.ActivationFunctionType.Tanh,
                                         bias=bg[:, l, k:k + 1])
            return g

        def fc_half(t, g_fc, half):
            """logits[:, half] = h1 @ fc_W_slice.T + fc_b_slice   (bf16)"""
            ps = fc_ps.tile([128, OS // 2], F32, tag="fcps", name="fps")
            for nck in range(2):
                fsl = bass.ts(half * 2 + nck, 512)   # slice into wfc
                nsl = bass.ts(nck, 512)              # slice into ps
                for k in range(KT):
                    nc.tensor.matmul(ps[:, nsl], g_fc[:, k, :], wfc[:, k, fsl],
                                     start=(k == 0), stop=(k == KT - 1))
            lsb = logp.tile([128, OS // 2], F32, tag="log", name="lsb")
            nc.vector.tensor_add(lsb[:], ps[:], fcbb[:, bass.ts(half, OS // 2)])
            nc.sync.dma_start(out_d[t][:, bass.ts(half, OS // 2)], lsb[:])

        g_fc_prev = None
        for t in range(n_steps):
            ps0 = rnn_ps.tile([128, H], F32, tag="rnnps", name="ps0")
            gemm_open(ps0, g_h0, w["hh0"])
            gemm_close(ps0, g_x, w["ih0"])

            ps1 = rnn_ps.tile([128, H], F32, tag="rnnps", name="ps1")
            gemm_open(ps1, g_h1, w["hh1"])   # PE: fills preact(h0) copy wait
            g_h0 = drain_layer(ps0, 0)
            if t > 0:
                fc_half(t - 1, g_fc_prev, 0)  # PE: fills tanh(h0) wait
            gemm_close(ps1, g_h0, w["ih1"])
            if t > 0:
                fc_half(t - 1, g_fc_prev, 1)  # PE: fills preact(h1)+tanh wait
            g_h1 = drain_layer(ps1, 1)
            g_x = g_h1
            g_fc = gfcp.tile([128, KT, BG], BF16, tag="gfc", name="gfc")
            nc.vector.tensor_copy(g_fc[:], g_h1[:])
            g_fc_prev = g_fc

        fc_half(n_steps - 1, g_fc_prev, 0)
        fc_half(n_steps - 1, g_fc_prev, 1)

    nc.finalize()
    return nc


def _prep_inputs(x, hidden, W_ih, W_hh, b_ih, b_hh, fc_W, fc_b, n_steps):
    """Build the 8 per-core input maps (host-side transposes)."""
    def gform(a):  # [BG, H] f32 -> [128, KT, BG]: out[p, k, b] = a[b, k*128+p]
        return np.ascontiguousarray(
            a.T.reshape(KT, 128, BG).transpose(1, 0, 2)).astype(np.float32)

    def wform(Wmat):  # [H_out, H_in] -> [128, KT, H_out] of W.T (f32)
        return np.ascontiguousarray(
            Wmat.T.reshape(KT, 128, Wmat.shape[0]).transpose(1, 0, 2)).astype(
                np.float32)

    common = {"ident": np.eye(128, dtype=np.float32)}
    for l, nm_pair in enumerate([("ih0", "hh0"), ("ih1", "hh1")]):
        for nm, Wmat in zip(nm_pair, (W_ih[l], W_hh[l])):
            common[f"w_{nm}"] = wform(Wmat)
    # bg[p, l, k] = (b_ih + b_hh)[l][k*128 + p]
    brows = (b_ih + b_hh).astype(np.float32)          # [L, H]
    common["bg"] = np.ascontiguousarray(
        brows.reshape(L, KT, 128).transpose(2, 0, 1))  # [128, L, KT]

    in_maps = []
    for c in range(N_CORES):
        bg_, j = c // 4, c % 4
        bsl = slice(bg_ * BG, (bg_ + 1) * BG)
        osl = slice(j * OS, (j + 1) * OS)
        wfc = np.ascontiguousarray(
            fc_W[osl].T.reshape(KT, 128, OS).transpose(1, 0, 2)).astype(
                ml_dtypes.bfloat16)
        m = dict(common)
        m["w_fc"] = wfc
        m["fcbb"] = np.ascontiguousarray(np.broadcast_to(
            fc_b[osl].astype(ml_dtypes.bfloat16), (128, OS)))
        for nm, src in (("x", x[0, bsl]), ("h0", hidden[0, bsl]),
                        ("h1", hidden[1, bsl])):
            m[f"g_{nm}"] = gform(src)
        in_maps.append(m)
    return in_maps


def kernel(x, hidden, embedded, W_ih, W_hh, b_ih, b_hh, fc_W, fc_b,
           _trace=False, _trace_kwargs=None):
    n_steps = embedded.shape[0]
    key = n_steps
    if key not in _cached:
        _cached[key] = _build_program(n_steps)
    nc = _cached[key]

    in_maps = _prep_inputs(np.asarray(x), np.asarray(hidden), np.asarray(W_ih),
                           np.asarray(W_hh), np.asarray(b_ih), np.asarray(b_hh),
                           np.asarray(fc_W), np.asarray(fc_b), n_steps)
    core_ids = list(range(N_CORES))
    res = run_bass_kernel_spmd(nc, in_maps, core_ids, trace=_trace,
                               **(_trace_kwargs or {}))

    out = np.empty((n_steps, 1, B, O), np.float32)
    for c in range(N_CORES):
        bg_, j = c // 4, c % 4
        out[:, 0, bg_ * BG:(bg_ + 1) * BG, j * OS:(j + 1) * OS] = \
            res.results[c]["out"]
    if _trace:
        kernel.last_results = res
    return out
